# revision 12
# baseline (speedup 1.0000x reference)
"""Trainium2 Bass kernel for the custom transformer layer.

Sharding: 8 cores = 4 batches x 2 query-row halves. Each core computes the
full layer for 512 query rows of one batch. K/V/ptm are computed for the
whole batch on both cores sharing it (duplicated; ~14% extra tensor-engine
work, but zero cross-core communication). Inputs are rotated per-core so the
own query rows always sit at positions 0..511 -> one SPMD program for all
cores.

All large matmuls run in bf16 with fp32 PSUM accumulation.  Softmax is
computed without max-subtraction (scores are bounded by construction), and
the ptm softmax / attention-softmax normalizations are folded in via
matmul-with-ones-column tricks so no partition-dim reductions are needed.
"""

import sys

sys.path.insert(0, "/opt/trn_rl_repo")

import numpy as np
import ml_dtypes

import concourse.bass as bass
import concourse.tile as tile
from concourse import bacc, mybir
from concourse.bass_utils import run_bass_kernel_spmd
from concourse.masks import make_identity

BF16 = ml_dtypes.bfloat16
F32 = mybir.dt.float32
BF = mybir.dt.bfloat16
AF = mybir.ActivationFunctionType
ALU = mybir.AluOpType

B, S, H, NH, DH, I, C, P = 4, 1024, 1280, 20, 64, 5120, 13, 8
SQ = 512          # query rows per core
KO = H // 128     # 10 k-chunks
IC = I // 128     # 40 i-chunks
LC = SQ // 128    # 4 l-chunks
MC = S // 128     # 8 m-chunks
LN_EPS = 1e-5


def bcast_ap(src: bass.AP, parts: int) -> bass.AP:
    """Partition-stride-0 broadcast AP (for DMA sources)."""
    return bass.AP(tensor=src.tensor, offset=src.offset,
                   ap=[[0, parts]] + [list(d) for d in src.ap])


def build_nc(use_mask: bool, bias_scale: float):
    nc = bacc.Bacc("TRN2", target_bir_lowering=False, debug=False, num_devices=8)

    # ---- DRAM parameters (per-core) ----
    h_d = nc.declare_dram_parameter("h", [S, H], F32, isOutput=False)
    hres_d = nc.declare_dram_parameter("hres", [SQ, H], F32, isOutput=False)
    wqT_d = nc.declare_dram_parameter("wqT", [H, H], BF, isOutput=False)
    wkT_d = nc.declare_dram_parameter("wkT", [H, H], BF, isOutput=False)
    wvT_d = nc.declare_dram_parameter("wvT", [H, H], BF, isOutput=False)
    wptmT_d = nc.declare_dram_parameter("wptmT", [H, P], BF, isOutput=False)
    rmat_d = nc.declare_dram_parameter("rmat", [P, P], BF, isOutput=False)
    wf1T_d = nc.declare_dram_parameter("wf1T", [H, I], BF, isOutput=False)
    wf2T_d = nc.declare_dram_parameter("wf2T", [I, H], BF, isOutput=False)
    bq_d = nc.declare_dram_parameter("bq", [KO, 128], F32, isOutput=False)
    bk_d = nc.declare_dram_parameter("bk", [KO, 128], F32, isOutput=False)
    bptm_d = nc.declare_dram_parameter("bptm", [P, 1], F32, isOutput=False)
    bf1_d = nc.declare_dram_parameter("bf1", [IC, 128], F32, isOutput=False)
    lng_d = nc.declare_dram_parameter("lng", [H], F32, isOutput=False)
    lnbf_d = nc.declare_dram_parameter("lnbf", [H], F32, isOutput=False)
    mb_d = nc.declare_dram_parameter("mb", [MC, 128], F32, isOutput=False)
    out_d = nc.declare_dram_parameter("out", [SQ, H], F32, isOutput=True)

    from contextlib import ExitStack
    with tile.TileContext(nc) as tc, ExitStack() as es:
        # ---- persistent constants ----
        const = es.enter_context(tc.tile_pool(name="const", bufs=1))
        ident_b = const.tile([128, 128], BF)
        make_identity(nc, ident_b)
        ident_f = const.tile([128, 128], F32)
        make_identity(nc, ident_f)
        bq_s = const.tile([128, KO], F32)
        nc.sync.dma_start(out=bq_s, in_=bq_d.ap().rearrange("c p -> p c"))
        bk_s = const.tile([128, KO], F32)
        nc.sync.dma_start(out=bk_s, in_=bk_d.ap().rearrange("c p -> p c"))
        bf1_s = const.tile([128, IC], F32)
        nc.sync.dma_start(out=bf1_s, in_=bf1_d.ap().rearrange("c p -> p c"))
        bptm_s = const.tile([P, 1], F32)
        nc.sync.dma_start(out=bptm_s, in_=bptm_d.ap())
        mb_s = const.tile([128, MC], F32)
        nc.sync.dma_start(out=mb_s, in_=mb_d.ap().rearrange("c p -> p c"))
        lng_b = const.tile([128, H], F32)
        nc.sync.dma_start(out=lng_b, in_=bcast_ap(lng_d.ap(), 128))
        lnbf_b = const.tile([128, H], F32)
        nc.sync.dma_start(out=lnbf_b, in_=bcast_ap(lnbf_d.ap(), 128))
        eps_s = const.tile([128, 1], F32)
        nc.vector.memset(eps_s, LN_EPS)
        ones8_s = const.tile([P, 1], F32)
        nc.vector.memset(ones8_s, 1.0)
        rmat_s = const.tile([P, P], BF)
        nc.sync.dma_start(out=rmat_s, in_=rmat_d.ap())
        wptm_s = const.tile([128, KO, P], BF)
        nc.sync.dma_start(out=wptm_s,
                          in_=wptmT_d.ap().rearrange("(ko p) c -> p ko c", p=128))

        # ---- lifetime-scoped persistent activations ----
        es_hT = ExitStack()       # phases A..C
        p_hT = es_hT.enter_context(tc.tile_pool(name="p_hT", bufs=1, side="right"))
        hT_s = p_hT.tile([128, KO, S], BF)             # h^T, feature-major
        es_attn = ExitStack()     # phases B..D
        p_attn = es_attn.enter_context(tc.tile_pool(name="p_attn", bufs=1))
        biasT_s = p_attn.tile([128, MC, SQ], BF)       # attention bias, [m, l]
        QT_s = p_attn.tile([128, KO, SQ], BF)          # (q/8)^T
        KT_s = p_attn.tile([128, KO, S], BF)           # k^T
        # V with a ones column per head: [s-part, s-chunk, head, 64+1]
        vaug_s = p_attn.tile([128, MC, NH, DH + 1], BF)

        # ================= Phase A: h -> hT (bf16) =================
        with tc.tile_pool(name="ph_a", bufs=3) as pa, \
             tc.tile_pool(name="ph_a_ps", bufs=4, space="PSUM") as paps:
            for sc in range(MC):
                hf = pa.tile([128, H], F32, tag="hf")
                nc.sync.dma_start(out=hf, in_=h_d.ap()[sc * 128:(sc + 1) * 128, :])
                hb = pa.tile([128, H], BF, tag="hb")
                nc.vector.tensor_copy(out=hb, in_=hf)
                for ko in range(KO):
                    tp = paps.tile([128, 128], BF)
                    nc.tensor.transpose(tp, hb[:, ko * 128:(ko + 1) * 128], ident_b)
                    nc.vector.tensor_copy(out=hT_s[:, ko, sc * 128:(sc + 1) * 128],
                                          in_=tp)

        # ================= Phase B: ptm -> biasT =================
        with tc.tile_pool(name="ph_b", bufs=2) as pb, \
             tc.tile_pool(name="ph_b_big", bufs=1) as pbb, \
             tc.tile_pool(name="ph_b_dram", bufs=1, space="DRAM") as pbd, \
             tc.tile_pool(name="ph_b_zps", bufs=2, space="PSUM") as pbzps, \
             tc.tile_pool(name="ph_b_ps", bufs=2, space="PSUM") as pbps:
            expT_s = pbb.tile([P, S], F32)
            rz = pb.tile([1, S], F32, tag="rz")
            for n2 in range(2):
                lp = pbps.tile([P, 512], F32, tag="logits")
                for ko in range(KO):
                    nc.tensor.matmul(lp, wptm_s[:, ko, :],
                                     hT_s[:, ko, n2 * 512:(n2 + 1) * 512],
                                     start=(ko == 0), stop=(ko == KO - 1))
                nc.scalar.activation(out=expT_s[:, n2 * 512:(n2 + 1) * 512],
                                     in_=lp, func=AF.Exp, bias=bptm_s)
            for n2 in range(2):
                zp = pbzps.tile([1, 512], F32, tag="z")
                nc.tensor.matmul(zp, ones8_s,
                                 expT_s[:, n2 * 512:(n2 + 1) * 512],
                                 start=True, stop=True)
                nc.vector.reciprocal(out=rz[:, n2 * 512:(n2 + 1) * 512], in_=zp)
            zscr = pbd.tile([1, S], F32)
            nc.sync.dma_start(out=zscr, in_=rz)
            zb = pbb.tile([P, S], F32)
            nc.sync.dma_start(out=zb, in_=bcast_ap(zscr[0, :], P))
            ptmT_s = pbb.tile([P, S], BF)
            nc.vector.tensor_mul(out=ptmT_s, in0=expT_s, in1=zb)
            gp = pbps.tile([P, 512], F32, tag="g")
            nc.tensor.matmul(gp, rmat_s, ptmT_s[:, :SQ], start=True, stop=True)
            gTs = pbb.tile([P, SQ], BF)
            nc.vector.tensor_copy(out=gTs, in_=gp)
            for mc in range(MC):
                up = pbps.tile([128, SQ], F32, tag="u")
                nc.tensor.matmul(up, ptmT_s[:, mc * 128:(mc + 1) * 128], gTs,
                                 start=True, stop=True)
                tt = pb.tile([128, SQ], F32, tag="tanh")
                nc.scalar.activation(out=tt, in_=up, func=AF.Tanh)
                if use_mask:
                    nc.vector.tensor_scalar(out=biasT_s[:, mc, :], in0=tt,
                                            scalar1=bias_scale,
                                            scalar2=mb_s[:, mc:mc + 1],
                                            op0=ALU.mult, op1=ALU.add)
                else:
                    nc.vector.tensor_scalar_mul(out=biasT_s[:, mc, :], in0=tt,
                                                scalar1=bias_scale)

        # ================= Phase C: QKV projections =================
        nc.vector.memset(vaug_s[:, :, :, DH:DH + 1], 1.0)
        with tc.tile_pool(name="ph_cq_w", bufs=1) as pcw, \
             tc.tile_pool(name="ph_cq_ps", bufs=4, space="PSUM") as pcps:
            wq_s = pcw.tile([128, KO, H], BF)
            nc.sync.dma_start(out=wq_s,
                              in_=wqT_d.ap().rearrange("(ko p) j -> p ko j", p=128))
            # Q^T (own 512 rows), scaled by 1/8
            for jc in range(KO):
                qp = pcps.tile([128, SQ], F32, tag="q")
                for ko in range(KO):
                    nc.tensor.matmul(qp, wq_s[:, ko, jc * 128:(jc + 1) * 128],
                                     hT_s[:, ko, :SQ],
                                     start=(ko == 0), stop=(ko == KO - 1))
                nc.scalar.activation(out=QT_s[:, jc, :], in_=qp, func=AF.Identity,
                                     bias=bq_s[:, jc:jc + 1], scale=0.125)
        with tc.tile_pool(name="ph_ck_w", bufs=1) as pcw2, \
             tc.tile_pool(name="ph_ck_ps", bufs=4, space="PSUM") as pcps2:
            wk_s = pcw2.tile([128, KO, H], BF)
            nc.sync.dma_start(out=wk_s,
                              in_=wkT_d.ap().rearrange("(ko p) j -> p ko j", p=128))
            # K^T (all 1024 rows)
            for jc in range(KO):
                for n2 in range(2):
                    kp = pcps2.tile([128, 512], F32, tag="k")
                    for ko in range(KO):
                        nc.tensor.matmul(kp, wk_s[:, ko, jc * 128:(jc + 1) * 128],
                                         hT_s[:, ko, n2 * 512:(n2 + 1) * 512],
                                         start=(ko == 0), stop=(ko == KO - 1))
                    nc.scalar.activation(out=KT_s[:, jc, n2 * 512:(n2 + 1) * 512],
                                         in_=kp, func=AF.Identity,
                                         bias=bk_s[:, jc:jc + 1])
        with tc.tile_pool(name="ph_cv_w", bufs=1) as pcw3, \
             tc.tile_pool(name="ph_cv_ps", bufs=4, space="PSUM") as pcps3:
            wv_s = pcw3.tile([128, KO, H], BF)
            nc.sync.dma_start(out=wv_s,
                              in_=wvT_d.ap().rearrange("(ko p) j -> p ko j", p=128))
            # V natural layout (all 1024 rows), written per-head with ones col
            for sc in range(MC):
                for j0, jn in ((0, 512), (512, 512), (1024, 256)):
                    vp = pcps3.tile([128, 512], F32, tag="v")
                    for ko in range(KO):
                        nc.tensor.matmul(vp[:, :jn],
                                         hT_s[:, ko, sc * 128:(sc + 1) * 128],
                                         wv_s[:, ko, j0:j0 + jn],
                                         start=(ko == 0), stop=(ko == KO - 1))
                    nc.scalar.activation(
                        out=vaug_s[:, sc, j0 // DH:(j0 + jn) // DH, 0:DH],
                        in_=vp[:, :jn].rearrange("p (h d) -> p h d", d=DH),
                        func=AF.Copy)
        es_hT.close()  # free h^T

        # ================= Phase D: attention =================
        es_ctx = ExitStack()      # phases D..E
        p_ctx = es_ctx.enter_context(tc.tile_pool(name="p_ctx", bufs=1, side="right"))
        ctxn_s = p_ctx.tile([128, LC, H], BF)          # attention out, natural
        with tc.tile_pool(name="ph_d", bufs=3) as pd, \
             tc.tile_pool(name="ph_d_pr", bufs=2) as pdp, \
             tc.tile_pool(name="ph_d_ps", bufs=3, space="PSUM") as pdps, \
             tc.tile_pool(name="ph_d_pst", bufs=2, space="PSUM") as pdpst, \
             tc.tile_pool(name="ph_d_ps2", bufs=2, space="PSUM") as pdps2:
            for hh in range(NH):
                ko, p0 = hh // 2, (hh % 2) * DH
                pt = pdp.tile([128, MC, SQ], BF, tag="probsT")
                for mc in range(MC):
                    sp = pdps.tile([128, SQ], F32, tag="sc")
                    nc.tensor.matmul(sp,
                                     KT_s[p0:p0 + DH, ko, mc * 128:(mc + 1) * 128],
                                     QT_s[p0:p0 + DH, ko, :],
                                     start=True, stop=True)
                    s2 = pd.tile([128, SQ], F32, tag="s2")
                    nc.vector.tensor_add(out=s2, in0=sp, in1=biasT_s[:, mc, :])
                    nc.scalar.activation(out=pt[:, mc, :], in_=s2, func=AF.Exp)
                cp = pdps2.tile([DH + 1, SQ], F32, tag="cx")
                for mc in range(MC):
                    nc.tensor.matmul(cp, vaug_s[:, mc, hh, :], pt[:, mc, :],
                                     start=(mc == 0), stop=(mc == MC - 1))
                cs = pd.tile([DH + 1, SQ], BF, tag="cs")
                nc.vector.tensor_copy(out=cs, in_=cp)
                for lc in range(LC):
                    tp = pdpst.tile([128, DH + 1], BF, tag="ct")
                    nc.tensor.transpose(tp, cs[:, lc * 128:(lc + 1) * 128],
                                        ident_b[:DH + 1, :DH + 1])
                    rc = pd.tile([128, 1], F32, tag="rc")
                    nc.vector.reciprocal(out=rc, in_=tp[:, DH:DH + 1])
                    nc.vector.tensor_scalar_mul(
                        out=ctxn_s[:, lc, hh * DH:(hh + 1) * DH],
                        in0=tp[:, 0:DH], scalar1=rc)

        es_attn.close()  # free biasT/QT/KT/V

        # ================= Phase E: residual + LN =================
        es_x = ExitStack()        # phases E..G
        p_x = es_x.enter_context(tc.tile_pool(name="p_x", bufs=1))
        x2_s = p_x.tile([128, LC, H], F32)             # LN out (+beta+bf2)
        xT_s = p_x.tile([128, KO, SQ], BF)             # x2^T
        gT_s = p_x.tile([128, IC, SQ], BF)             # gelu(ffn1)^T
        with tc.tile_pool(name="ph_e", bufs=2) as pe, \
             tc.tile_pool(name="ph_e_ps", bufs=4, space="PSUM") as peps:
            for lc in range(LC):
                hr = pe.tile([128, H], F32, tag="hr")
                nc.sync.dma_start(out=hr,
                                  in_=hres_d.ap()[lc * 128:(lc + 1) * 128, :])
                xs = x2_s[:, lc, :]
                nc.vector.tensor_add(out=xs, in0=hr, in1=ctxn_s[:, lc, :])
                st = pe.tile([128, 5, 6], F32, tag="st")
                xg = xs.rearrange("p (g d) -> p g d", d=256)
                for sg in range(5):
                    nc.vector.bn_stats(out=st[:, sg, :], in_=xg[:, sg, :])
                mv = pe.tile([128, 2], F32, tag="mv")
                nc.vector.bn_aggr(out=mv, in_=st)
                sd = pe.tile([128, 1], F32, tag="sd")
                nc.scalar.activation(out=sd, in_=mv[:, 1:2], func=AF.Sqrt,
                                     bias=eps_s)
                rs = pe.tile([128, 1], F32, tag="rs")
                nc.vector.reciprocal(out=rs, in_=sd)
                nc.vector.tensor_scalar(out=xs, in0=xs, scalar1=mv[:, 0:1],
                                        scalar2=rs, op0=ALU.subtract, op1=ALU.mult)
                nc.vector.tensor_mul(out=xs, in0=xs, in1=lng_b)
                nc.vector.tensor_add(out=xs, in0=xs, in1=lnbf_b)
                for ko in range(KO):
                    tp = peps.tile([128, 128], F32, tag="xt")
                    nc.tensor.transpose(tp, xs[:, ko * 128:(ko + 1) * 128], ident_f)
                    nc.vector.tensor_copy(
                        out=xT_s[:, ko, lc * 128:(lc + 1) * 128], in_=tp)
        es_ctx.close()  # free ctxn

        # ================= Phase F: FFN1 (gelu) =================
        with tc.tile_pool(name="ph_f_w", bufs=3) as pfw, \
             tc.tile_pool(name="ph_f_ps", bufs=4, space="PSUM") as pfps:
            for ic2 in range(IC // 2):
                wt = pfw.tile([128, KO, 256], BF, tag="w1")
                nc.sync.dma_start(
                    out=wt,
                    in_=wf1T_d.ap()[:, ic2 * 256:(ic2 + 1) * 256]
                        .rearrange("(ko p) i -> p ko i", p=128))
                for i_in in range(2):
                    ic = ic2 * 2 + i_in
                    yp = pfps.tile([128, SQ], F32, tag="y")
                    for ko in range(KO):
                        nc.tensor.matmul(yp,
                                         wt[:, ko, i_in * 128:(i_in + 1) * 128],
                                         xT_s[:, ko, :],
                                         start=(ko == 0), stop=(ko == KO - 1))
                    nc.scalar.activation(out=gT_s[:, ic, :], in_=yp, func=AF.Gelu,
                                         bias=bf1_s[:, ic:ic + 1])

        # ================= Phase G: FFN2 + residual + store =================
        with tc.tile_pool(name="ph_g_w", bufs=3) as pgw, \
             tc.tile_pool(name="ph_g_o", bufs=3) as pgo, \
             tc.tile_pool(name="ph_g_ps", bufs=1, space="PSUM") as pgps:
            for j0, jn in ((0, 512), (512, 512), (1024, 256)):
                zps = [pgps.tile([128, jn], F32, tag=f"z{lc}", name=f"zp_{j0}_{lc}")
                       for lc in range(LC)]
                for ic in range(IC):
                    w2 = pgw.tile([128, 512], BF, tag="w2")
                    nc.sync.dma_start(out=w2[:, :jn],
                                      in_=wf2T_d.ap()[ic * 128:(ic + 1) * 128,
                                                      j0:j0 + jn])
                    for lc in range(LC):
                        nc.tensor.matmul(zps[lc],
                                         gT_s[:, ic, lc * 128:(lc + 1) * 128],
                                         w2[:, :jn],
                                         start=(ic == 0), stop=(ic == IC - 1))
                for lc in range(LC):
                    ot = pgo.tile([128, 512], F32, tag="ot")
                    nc.vector.tensor_add(out=ot[:, :jn], in0=zps[lc],
                                         in1=x2_s[:, lc, j0:j0 + jn])
                    nc.sync.dma_start(
                        out=out_d.ap()[lc * 128:(lc + 1) * 128, j0:j0 + jn],
                        in_=ot[:, :jn])
        es_x.close()

    nc.compile()
    return nc


_NC_CACHE = {}


def _get_nc(use_mask: bool, bias_scale: float):
    key = (use_mask, round(bias_scale, 9))
    if key not in _NC_CACHE:
        _NC_CACHE[key] = build_nc(use_mask, bias_scale)
    return _NC_CACHE[key]


def _prep_inputs(inputs):
    f32 = lambda x: np.ascontiguousarray(np.asarray(x, np.float32))
    bft = lambda x: np.ascontiguousarray(np.asarray(x, np.float32).T).astype(BF16)
    hs = f32(inputs["hidden_states"])
    mask = f32(inputs["attention_mask"])
    M, W1, b1, W2, b2 = (f32(inputs["M"]), f32(inputs["W_ct1"]),
                         f32(inputs["b_ct1"]), f32(inputs["W_ct2"]),
                         f32(inputs["b_ct2"]))
    R = ((M.T @ W1.T + b1).T @ (M @ W2.T + b2)).astype(np.float32)
    bias_scale = float(np.asarray(inputs["bias_scale"]).reshape(-1)[0])
    use_mask = not bool(np.all(mask == 1.0))

    shared = {
        "wqT": bft(inputs["Wq"]), "wkT": bft(inputs["Wk"]),
        "wvT": bft(inputs["Wv"]), "wptmT": bft(inputs["W_ptm"]),
        "rmat": np.ascontiguousarray(R).astype(BF16),
        "wf1T": bft(inputs["Wf1"]), "wf2T": bft(inputs["Wf2"]),
        "bq": f32(inputs["bq"]).reshape(KO, 128),
        "bk": f32(inputs["bk"]).reshape(KO, 128),
        "bptm": f32(inputs["b_ptm"]).reshape(P, 1),
        "bf1": f32(inputs["bf1"]).reshape(IC, 128),
        "lng": f32(inputs["ln_g"]),
        "lnbf": f32(inputs["ln_b"]) + f32(inputs["bf2"]),
    }
    bv = f32(inputs["bv"])
    in_maps = []
    for c in range(8):
        b, half = c // 2, c % 2
        r0 = half * SQ
        mb = np.roll((1.0 - mask[b]) * np.float32(-1e30), -r0)
        m = dict(shared)
        m["h"] = np.ascontiguousarray(np.roll(hs[b], -r0, axis=0))
        m["hres"] = np.ascontiguousarray(hs[b, r0:r0 + SQ] + bv[None, :])
        m["mb"] = np.ascontiguousarray(mb.reshape(MC, 128))
        in_maps.append(m)
    return in_maps, use_mask, bias_scale


def kernel(**inputs) -> np.ndarray:
    in_maps, use_mask, bias_scale = _prep_inputs(inputs)
    nc = _get_nc(use_mask, bias_scale)
    res = run_bass_kernel_spmd(nc, in_maps, list(range(8)))
    out = np.zeros((B, S, H), np.float32)
    for c in range(8):
        b, half = c // 2, c % 2
        r0 = half * SQ
        out[b, r0:r0 + SQ] = res.results[c]["out"]
    return out


# revision 13
# speedup vs baseline: 1.1929x; 1.1929x over previous
"""Trainium2 Bass kernel for the custom transformer layer.

Sharding: 8 cores = 4 batches x 2 query-row halves. Each core computes the
full layer for 512 query rows of one batch. K/V/ptm are computed for the
whole batch on both cores sharing it (duplicated; ~14% extra tensor-engine
work, but zero cross-core communication). Inputs are rotated per-core so the
own query rows always sit at positions 0..511 -> one SPMD program for all
cores.

All large matmuls run in bf16 with fp32 PSUM accumulation.  Softmax is
computed without max-subtraction (scores are bounded by construction), and
the ptm softmax / attention-softmax normalizations are folded in via
matmul-with-ones-column tricks so no partition-dim reductions are needed.
"""

import sys

sys.path.insert(0, "/opt/trn_rl_repo")

import numpy as np
import ml_dtypes

import concourse.bass as bass
import concourse.tile as tile
from concourse import bacc, mybir
from concourse.bass_utils import run_bass_kernel_spmd
from concourse.masks import make_identity

BF16 = ml_dtypes.bfloat16
F32 = mybir.dt.float32
BF = mybir.dt.bfloat16
AF = mybir.ActivationFunctionType
ALU = mybir.AluOpType

B, S, H, NH, DH, I, C, P = 4, 1024, 1280, 20, 64, 5120, 13, 8
SQ = 512          # query rows per core
KO = H // 128     # 10 k-chunks
IC = I // 128     # 40 i-chunks
LC = SQ // 128    # 4 l-chunks
MC = S // 128     # 8 m-chunks
LN_EPS = 1e-5


def bcast_ap(src: bass.AP, parts: int) -> bass.AP:
    """Partition-stride-0 broadcast AP (for DMA sources)."""
    return bass.AP(tensor=src.tensor, offset=src.offset,
                   ap=[[0, parts]] + [list(d) for d in src.ap])


def build_nc(use_mask: bool, bias_scale: float):
    nc = bacc.Bacc("TRN2", target_bir_lowering=False, debug=False, num_devices=8)

    # ---- DRAM parameters (per-core) ----
    h_d = nc.declare_dram_parameter("h", [S, H], F32, isOutput=False)
    hres_d = nc.declare_dram_parameter("hres", [SQ, H], F32, isOutput=False)
    wqT_d = nc.declare_dram_parameter("wqT", [128, KO, H], BF, isOutput=False)
    wkT_d = nc.declare_dram_parameter("wkT", [128, KO, H], BF, isOutput=False)
    wvT_d = nc.declare_dram_parameter("wvT", [128, KO, H], BF, isOutput=False)
    wptmT_d = nc.declare_dram_parameter("wptmT", [H, P], BF, isOutput=False)
    rmat_d = nc.declare_dram_parameter("rmat", [P, P], BF, isOutput=False)
    wf1T_d = nc.declare_dram_parameter("wf1T", [IC // 2, 128, KO, 256], BF,
                                       isOutput=False)
    wf2T_d = nc.declare_dram_parameter("wf2T", [IC, 128, H], BF, isOutput=False)
    bq_d = nc.declare_dram_parameter("bq", [KO, 128], F32, isOutput=False)
    bk_d = nc.declare_dram_parameter("bk", [KO, 128], F32, isOutput=False)
    bptm_d = nc.declare_dram_parameter("bptm", [P, 1], F32, isOutput=False)
    bf1_d = nc.declare_dram_parameter("bf1", [IC, 128], F32, isOutput=False)
    lng_d = nc.declare_dram_parameter("lng", [H], F32, isOutput=False)
    lnbf_d = nc.declare_dram_parameter("lnbf", [H], F32, isOutput=False)
    mb_d = nc.declare_dram_parameter("mb", [MC, 128], F32, isOutput=False)
    out_d = nc.declare_dram_parameter("out", [SQ, H], F32, isOutput=True)

    from contextlib import ExitStack
    with tile.TileContext(nc) as tc, ExitStack() as es:
        # ---- persistent constants ----
        const = es.enter_context(tc.tile_pool(name="const", bufs=1))
        ident_b = const.tile([128, 128], BF)
        make_identity(nc, ident_b)
        ident_f = const.tile([128, 128], F32)
        make_identity(nc, ident_f)
        bq_s = const.tile([128, KO], F32)
        nc.sync.dma_start(out=bq_s, in_=bq_d.ap().rearrange("c p -> p c"))
        bk_s = const.tile([128, KO], F32)
        nc.sync.dma_start(out=bk_s, in_=bk_d.ap().rearrange("c p -> p c"))
        bf1_s = const.tile([128, IC], F32)
        nc.sync.dma_start(out=bf1_s, in_=bf1_d.ap().rearrange("c p -> p c"))
        bptm_s = const.tile([P, 1], F32)
        nc.sync.dma_start(out=bptm_s, in_=bptm_d.ap())
        mb_s = const.tile([128, MC], F32)
        nc.sync.dma_start(out=mb_s, in_=mb_d.ap().rearrange("c p -> p c"))
        lng_b = const.tile([128, H], F32)
        nc.sync.dma_start(out=lng_b, in_=bcast_ap(lng_d.ap(), 128))
        lnbf_b = const.tile([128, H], F32)
        nc.sync.dma_start(out=lnbf_b, in_=bcast_ap(lnbf_d.ap(), 128))
        eps_s = const.tile([128, 1], F32)
        nc.vector.memset(eps_s, LN_EPS)
        ones8_s = const.tile([P, 1], F32)
        nc.vector.memset(ones8_s, 1.0)
        rmat_s = const.tile([P, P], BF)
        nc.sync.dma_start(out=rmat_s, in_=rmat_d.ap())
        wptm_s = const.tile([128, KO, P], BF)
        nc.sync.dma_start(out=wptm_s,
                          in_=wptmT_d.ap().rearrange("(ko p) c -> p ko c", p=128))

        # ---- lifetime-scoped persistent activations ----
        es_w = ExitStack()        # QKV weights, prefetched from t=0
        p_w = es_w.enter_context(tc.tile_pool(name="p_w", bufs=1, side="right"))
        wq_s = p_w.tile([128, KO, H], BF)
        wk_s = p_w.tile([128, KO, H], BF)
        wv_s = p_w.tile([128, KO, H], BF)
        for wt_s, wt_d in ((wq_s, wqT_d), (wk_s, wkT_d), (wv_s, wvT_d)):
            nc.sync.dma_start(out=wt_s[:, 0:KO // 2, :],
                              in_=wt_d.ap()[:, 0:KO // 2, :])
            nc.gpsimd.dma_start(out=wt_s[:, KO // 2:KO, :],
                                in_=wt_d.ap()[:, KO // 2:KO, :])
        es_hT = ExitStack()       # phases A..C
        p_hT = es_hT.enter_context(tc.tile_pool(name="p_hT", bufs=1, side="right"))
        hT_s = p_hT.tile([128, KO, S], BF)             # h^T, feature-major
        es_attn = ExitStack()     # phases B..D
        p_attn = es_attn.enter_context(tc.tile_pool(name="p_attn", bufs=1))
        biasT_s = p_attn.tile([128, MC, SQ], BF)       # attention bias, [m, l]
        QT_s = p_attn.tile([128, KO, SQ], BF)          # (q/8)^T
        KT_s = p_attn.tile([128, KO, S], BF)           # k^T
        # V with a ones column per head: [s-part, s-chunk, head, 64+1]
        vaug_s = p_attn.tile([128, MC, NH, DH + 1], BF)

        # ================= Phase A: h -> hT (bf16) =================
        with tc.tile_pool(name="ph_a", bufs=3) as pa, \
             tc.tile_pool(name="ph_a_ps", bufs=4, space="PSUM") as paps:
            for sc in range(MC):
                hf = pa.tile([128, H], F32, tag="hf")
                nc.sync.dma_start(out=hf, in_=h_d.ap()[sc * 128:(sc + 1) * 128, :])
                hb = pa.tile([128, H], BF, tag="hb")
                nc.vector.tensor_copy(out=hb, in_=hf)
                for ko in range(KO):
                    tp = paps.tile([128, 128], BF)
                    nc.tensor.transpose(tp, hb[:, ko * 128:(ko + 1) * 128], ident_b)
                    nc.vector.tensor_copy(out=hT_s[:, ko, sc * 128:(sc + 1) * 128],
                                          in_=tp)

        # ================= Phase B: ptm -> biasT =================
        with tc.tile_pool(name="ph_b", bufs=2) as pb, \
             tc.tile_pool(name="ph_b_big", bufs=1) as pbb, \
             tc.tile_pool(name="ph_b_dram", bufs=1, space="DRAM") as pbd, \
             tc.tile_pool(name="ph_b_zps", bufs=2, space="PSUM") as pbzps, \
             tc.tile_pool(name="ph_b_ps", bufs=2, space="PSUM") as pbps:
            expT_s = pbb.tile([P, S], F32)
            rz = pb.tile([1, S], F32, tag="rz")
            for n2 in range(2):
                lp = pbps.tile([P, 512], F32, tag="logits")
                for ko in range(KO):
                    nc.tensor.matmul(lp, wptm_s[:, ko, :],
                                     hT_s[:, ko, n2 * 512:(n2 + 1) * 512],
                                     start=(ko == 0), stop=(ko == KO - 1))
                nc.scalar.activation(out=expT_s[:, n2 * 512:(n2 + 1) * 512],
                                     in_=lp, func=AF.Exp, bias=bptm_s)
            for n2 in range(2):
                zp = pbzps.tile([1, 512], F32, tag="z")
                nc.tensor.matmul(zp, ones8_s,
                                 expT_s[:, n2 * 512:(n2 + 1) * 512],
                                 start=True, stop=True)
                nc.vector.reciprocal(out=rz[:, n2 * 512:(n2 + 1) * 512], in_=zp)
            zscr = pbd.tile([1, S], F32)
            nc.sync.dma_start(out=zscr, in_=rz)
            zb = pbb.tile([P, S], F32)
            nc.sync.dma_start(out=zb, in_=bcast_ap(zscr[0, :], P))
            ptmT_s = pbb.tile([P, S], BF)
            nc.vector.tensor_mul(out=ptmT_s, in0=expT_s, in1=zb)
            gp = pbps.tile([P, 512], F32, tag="g")
            nc.tensor.matmul(gp, rmat_s, ptmT_s[:, :SQ], start=True, stop=True)
            gTs = pbb.tile([P, SQ], BF)
            nc.vector.tensor_copy(out=gTs, in_=gp)
            for mc in range(MC):
                up = pbps.tile([128, SQ], F32, tag="u")
                nc.tensor.matmul(up, ptmT_s[:, mc * 128:(mc + 1) * 128], gTs,
                                 start=True, stop=True)
                tt = pb.tile([128, SQ], F32, tag="tanh")
                nc.scalar.activation(out=tt, in_=up, func=AF.Tanh)
                if use_mask:
                    nc.vector.tensor_scalar(out=biasT_s[:, mc, :], in0=tt,
                                            scalar1=bias_scale,
                                            scalar2=mb_s[:, mc:mc + 1],
                                            op0=ALU.mult, op1=ALU.add)
                else:
                    nc.vector.tensor_scalar_mul(out=biasT_s[:, mc, :], in0=tt,
                                                scalar1=bias_scale)

        # ================= Phase C: QKV projections =================
        nc.vector.memset(vaug_s[:, :, :, DH:DH + 1], 1.0)
        with tc.tile_pool(name="ph_cq_ps", bufs=4, space="PSUM") as pcps:
            # Q^T (own 512 rows), scaled by 1/8
            for jc in range(KO):
                qp = pcps.tile([128, SQ], F32, tag="q")
                for ko in range(KO):
                    nc.tensor.matmul(qp, wq_s[:, ko, jc * 128:(jc + 1) * 128],
                                     hT_s[:, ko, :SQ],
                                     start=(ko == 0), stop=(ko == KO - 1))
                nc.scalar.activation(out=QT_s[:, jc, :], in_=qp, func=AF.Identity,
                                     bias=bq_s[:, jc:jc + 1], scale=0.125)
        with tc.tile_pool(name="ph_ck_ps", bufs=4, space="PSUM") as pcps2:
            # K^T (all 1024 rows)
            for jc in range(KO):
                for n2 in range(2):
                    kp = pcps2.tile([128, 512], F32, tag="k")
                    for ko in range(KO):
                        nc.tensor.matmul(kp, wk_s[:, ko, jc * 128:(jc + 1) * 128],
                                         hT_s[:, ko, n2 * 512:(n2 + 1) * 512],
                                         start=(ko == 0), stop=(ko == KO - 1))
                    nc.scalar.activation(out=KT_s[:, jc, n2 * 512:(n2 + 1) * 512],
                                         in_=kp, func=AF.Identity,
                                         bias=bk_s[:, jc:jc + 1])
        with tc.tile_pool(name="ph_cv_ps", bufs=4, space="PSUM") as pcps3:
            # V natural layout (all 1024 rows), written per-head with ones col
            for sc in range(MC):
                for j0, jn in ((0, 512), (512, 512), (1024, 256)):
                    vp = pcps3.tile([128, 512], F32, tag="v")
                    for ko in range(KO):
                        nc.tensor.matmul(vp[:, :jn],
                                         hT_s[:, ko, sc * 128:(sc + 1) * 128],
                                         wv_s[:, ko, j0:j0 + jn],
                                         start=(ko == 0), stop=(ko == KO - 1))
                    nc.scalar.activation(
                        out=vaug_s[:, sc, j0 // DH:(j0 + jn) // DH, 0:DH],
                        in_=vp[:, :jn].rearrange("p (h d) -> p h d", d=DH),
                        func=AF.Copy)
        es_hT.close()  # free h^T
        es_w.close()   # free QKV weights

        # ================= Phase D: attention =================
        es_ctx = ExitStack()      # phases D..E
        p_ctx = es_ctx.enter_context(tc.tile_pool(name="p_ctx", bufs=1, side="right"))
        ctxn_s = p_ctx.tile([128, LC, H], BF)          # attention out, natural
        with tc.tile_pool(name="ph_d", bufs=3) as pd, \
             tc.tile_pool(name="ph_d_pr", bufs=2) as pdp, \
             tc.tile_pool(name="ph_d_ps", bufs=3, space="PSUM") as pdps, \
             tc.tile_pool(name="ph_d_pst", bufs=2, space="PSUM") as pdpst, \
             tc.tile_pool(name="ph_d_ps2", bufs=2, space="PSUM") as pdps2:
            for hh in range(NH):
                ko, p0 = hh // 2, (hh % 2) * DH
                pt = pdp.tile([128, MC, SQ], BF, tag="probsT")
                for mc in range(MC):
                    sp = pdps.tile([128, SQ], F32, tag="sc")
                    nc.tensor.matmul(sp,
                                     KT_s[p0:p0 + DH, ko, mc * 128:(mc + 1) * 128],
                                     QT_s[p0:p0 + DH, ko, :],
                                     start=True, stop=False)
                    nc.tensor.matmul(sp, ident_b, biasT_s[:, mc, :],
                                     start=False, stop=True)
                    nc.scalar.activation(out=pt[:, mc, :], in_=sp, func=AF.Exp)
                cp = pdps2.tile([DH + 1, SQ], F32, tag="cx")
                for mc in range(MC):
                    nc.tensor.matmul(cp, vaug_s[:, mc, hh, :], pt[:, mc, :],
                                     start=(mc == 0), stop=(mc == MC - 1))
                cs = pd.tile([DH + 1, SQ], BF, tag="cs")
                nc.vector.tensor_copy(out=cs, in_=cp)
                for lc in range(LC):
                    tp = pdpst.tile([128, DH + 1], BF, tag="ct")
                    nc.tensor.transpose(tp, cs[:, lc * 128:(lc + 1) * 128],
                                        ident_b[:DH + 1, :DH + 1])
                    rc = pd.tile([128, 1], F32, tag="rc")
                    nc.vector.reciprocal(out=rc, in_=tp[:, DH:DH + 1])
                    nc.vector.tensor_scalar_mul(
                        out=ctxn_s[:, lc, hh * DH:(hh + 1) * DH],
                        in0=tp[:, 0:DH], scalar1=rc)

        es_attn.close()  # free biasT/QT/KT/V

        # ================= Phase E: residual + LN =================
        es_x = ExitStack()        # phases E..G
        p_x = es_x.enter_context(tc.tile_pool(name="p_x", bufs=1))
        x2_s = p_x.tile([128, LC, H], F32)             # LN out (+beta+bf2)
        xT_s = p_x.tile([128, KO, SQ], BF)             # x2^T
        gT_s = p_x.tile([128, IC, SQ], BF)             # gelu(ffn1)^T
        with tc.tile_pool(name="ph_e", bufs=2) as pe, \
             tc.tile_pool(name="ph_e_ps", bufs=4, space="PSUM") as peps:
            for lc in range(LC):
                hr = pe.tile([128, H], F32, tag="hr")
                nc.sync.dma_start(out=hr,
                                  in_=hres_d.ap()[lc * 128:(lc + 1) * 128, :])
                xs = x2_s[:, lc, :]
                nc.vector.tensor_add(out=xs, in0=hr, in1=ctxn_s[:, lc, :])
                st = pe.tile([128, 5, 6], F32, tag="st")
                xg = xs.rearrange("p (g d) -> p g d", d=256)
                for sg in range(5):
                    nc.vector.bn_stats(out=st[:, sg, :], in_=xg[:, sg, :])
                mv = pe.tile([128, 2], F32, tag="mv")
                nc.vector.bn_aggr(out=mv, in_=st)
                sd = pe.tile([128, 1], F32, tag="sd")
                nc.scalar.activation(out=sd, in_=mv[:, 1:2], func=AF.Sqrt,
                                     bias=eps_s)
                rs = pe.tile([128, 1], F32, tag="rs")
                nc.vector.reciprocal(out=rs, in_=sd)
                nc.vector.tensor_scalar(out=xs, in0=xs, scalar1=mv[:, 0:1],
                                        scalar2=rs, op0=ALU.subtract, op1=ALU.mult)
                nc.vector.tensor_mul(out=xs, in0=xs, in1=lng_b)
                nc.vector.tensor_add(out=xs, in0=xs, in1=lnbf_b)
                for ko in range(KO):
                    tp = peps.tile([128, 128], F32, tag="xt")
                    nc.tensor.transpose(tp, xs[:, ko * 128:(ko + 1) * 128], ident_f)
                    nc.vector.tensor_copy(
                        out=xT_s[:, ko, lc * 128:(lc + 1) * 128], in_=tp)
        es_ctx.close()  # free ctxn

        # ================= Phase F: FFN1 (gelu) =================
        with tc.tile_pool(name="ph_f_w", bufs=4) as pfw, \
             tc.tile_pool(name="ph_f_ps", bufs=4, space="PSUM") as pfps:
            for ic2 in range(IC // 2):
                wt = pfw.tile([128, KO, 256], BF, tag="w1")
                eng = nc.sync if ic2 % 2 == 0 else nc.gpsimd
                eng.dma_start(out=wt, in_=wf1T_d.ap()[ic2])
                for i_in in range(2):
                    ic = ic2 * 2 + i_in
                    yp = pfps.tile([128, SQ], F32, tag="y")
                    for ko in range(KO):
                        nc.tensor.matmul(yp,
                                         wt[:, ko, i_in * 128:(i_in + 1) * 128],
                                         xT_s[:, ko, :],
                                         start=(ko == 0), stop=(ko == KO - 1))
                    nc.scalar.activation(out=gT_s[:, ic, :], in_=yp, func=AF.Gelu,
                                         bias=bf1_s[:, ic:ic + 1])

        # ================= Phase G: FFN2 + residual + store =================
        with tc.tile_pool(name="ph_g_w", bufs=5) as pgw, \
             tc.tile_pool(name="ph_g_o", bufs=3) as pgo, \
             tc.tile_pool(name="ph_g_ps", bufs=1, space="PSUM") as pgps:
            for j0, jn in ((0, 512), (512, 512), (1024, 256)):
                zps = [pgps.tile([128, jn], F32, tag=f"z{lc}", name=f"zp_{j0}_{lc}")
                       for lc in range(LC)]
                for ic in range(IC):
                    w2 = pgw.tile([128, 512], BF, tag="w2")
                    eng = nc.sync if ic % 2 == 0 else nc.gpsimd
                    eng.dma_start(out=w2[:, :jn],
                                  in_=wf2T_d.ap()[ic, :, j0:j0 + jn])
                    for lc in range(LC):
                        nc.tensor.matmul(zps[lc],
                                         gT_s[:, ic, lc * 128:(lc + 1) * 128],
                                         w2[:, :jn],
                                         start=(ic == 0), stop=(ic == IC - 1))
                for lc in range(LC):
                    ot = pgo.tile([128, 512], F32, tag="ot")
                    nc.vector.tensor_add(out=ot[:, :jn], in0=zps[lc],
                                         in1=x2_s[:, lc, j0:j0 + jn])
                    nc.sync.dma_start(
                        out=out_d.ap()[lc * 128:(lc + 1) * 128, j0:j0 + jn],
                        in_=ot[:, :jn])
        es_x.close()

    nc.compile()
    return nc


_NC_CACHE = {}


def _get_nc(use_mask: bool, bias_scale: float):
    key = (use_mask, round(bias_scale, 9))
    if key not in _NC_CACHE:
        _NC_CACHE[key] = build_nc(use_mask, bias_scale)
    return _NC_CACHE[key]


def _prep_inputs(inputs):
    f32 = lambda x: np.ascontiguousarray(np.asarray(x, np.float32))
    bft = lambda x: np.ascontiguousarray(np.asarray(x, np.float32).T).astype(BF16)
    hs = f32(inputs["hidden_states"])
    mask = f32(inputs["attention_mask"])
    M, W1, b1, W2, b2 = (f32(inputs["M"]), f32(inputs["W_ct1"]),
                         f32(inputs["b_ct1"]), f32(inputs["W_ct2"]),
                         f32(inputs["b_ct2"]))
    R = ((M.T @ W1.T + b1).T @ (M @ W2.T + b2)).astype(np.float32)
    bias_scale = float(np.asarray(inputs["bias_scale"]).reshape(-1)[0])
    use_mask = not bool(np.all(mask == 1.0))

    def pack_kxj(wT):
        # (H, J) -> (128, KO, J): partition-major, contiguous per partition
        return np.ascontiguousarray(
            wT.reshape(KO, 128, wT.shape[1]).transpose(1, 0, 2))

    wf1T = bft(inputs["Wf1"])                     # (H, I)
    wf1p = np.ascontiguousarray(
        wf1T.reshape(KO, 128, IC // 2, 256).transpose(2, 0, 1, 3)
            .transpose(0, 2, 1, 3))               # (IC//2, 128, KO, 256)
    wf2T = bft(inputs["Wf2"])                     # (I, H)
    wf2p = np.ascontiguousarray(wf2T.reshape(IC, 128, H))
    shared = {
        "wqT": pack_kxj(bft(inputs["Wq"])), "wkT": pack_kxj(bft(inputs["Wk"])),
        "wvT": pack_kxj(bft(inputs["Wv"])), "wptmT": bft(inputs["W_ptm"]),
        "rmat": np.ascontiguousarray(R).astype(BF16),
        "wf1T": wf1p, "wf2T": wf2p,
        "bq": f32(inputs["bq"]).reshape(KO, 128),
        "bk": f32(inputs["bk"]).reshape(KO, 128),
        "bptm": f32(inputs["b_ptm"]).reshape(P, 1),
        "bf1": f32(inputs["bf1"]).reshape(IC, 128),
        "lng": f32(inputs["ln_g"]),
        "lnbf": f32(inputs["ln_b"]) + f32(inputs["bf2"]),
    }
    bv = f32(inputs["bv"])
    in_maps = []
    for c in range(8):
        b, half = c // 2, c % 2
        r0 = half * SQ
        mb = np.roll((1.0 - mask[b]) * np.float32(-1e30), -r0)
        m = dict(shared)
        m["h"] = np.ascontiguousarray(np.roll(hs[b], -r0, axis=0))
        m["hres"] = np.ascontiguousarray(hs[b, r0:r0 + SQ] + bv[None, :])
        m["mb"] = np.ascontiguousarray(mb.reshape(MC, 128))
        in_maps.append(m)
    return in_maps, use_mask, bias_scale


def kernel(**inputs) -> np.ndarray:
    in_maps, use_mask, bias_scale = _prep_inputs(inputs)
    nc = _get_nc(use_mask, bias_scale)
    res = run_bass_kernel_spmd(nc, in_maps, list(range(8)))
    out = np.zeros((B, S, H), np.float32)
    for c in range(8):
        b, half = c // 2, c % 2
        r0 = half * SQ
        out[b, r0:r0 + SQ] = res.results[c]["out"]
    return out
